# revision 6
# baseline (speedup 1.0000x reference)
"""Multi-head attention kernel for 8 Trainium2 NeuronCores.

Problem: B=2, S=2048, H=8, DK=DV=64, D=512 (nn_MultiHeadAttention).

Sharding: core c owns batch b=c//4 and query rows [512*r, 512*r+512),
r = c%4. No collectives: every core computes the K/V projections for all
8 heads locally (K/V proj is cheap at full PE clock; the 4-way AllGather
it would replace costs ~57us and stalls the PE).

Per-core device kernel (heads processed as 4 pairs of 2):
  KT[p]  = wk[:,pair p].T @ kT + bk      [128, 2048] fp16
  QT[p]  = wq[:,pair p].T @ qT + bq      [128, 512]  fp16
  V'[t]  = vT[t].T @ wv + bv | 1         [128, 8, 65] fp16 (ones col
           makes the ov matmul emit the softmax denominator in row 64)
  scores pair-step: two C=64 matmuls at tile_position (0,0)/(64,0) run
           concurrently on the PE array halves -> [128, 2, 512] psum
  attnT  = exp(scores/8) on ScalarE, fp16, no max-subtract
  o65   += V'[t,h].T @ attnT[h], accumulated over t (rows 0:64 = head
           output, row 64 = denominator)
  o2T[p] = o65[0:64] * (1/o65[64]) via DVE recip + GPSIMD
           partition_broadcast + DVE mul, packed per pair [128, 512]
  out    = sum_p o2T[p].T-chunks @ wo2[p] + bo   (C=128 pair-stacked)

Projection matmuls are interleaved into the attention windows so the PE
never idles (DVFS keeps the 2.4GHz clock only while the PE is dense).
"""

import numpy as np

B, S, H, DK, DV = 2, 2048, 8, 64, 64
D = H * DV  # 512
NCORES = 8
ROWS = (B * S) // NCORES  # 512 query rows per core
NPAIR = H // 2  # 4 head pairs
NTT = S // 128  # 16 key/value tiles
NDC = D // 128  # 4 contraction chunks
P = 128
VW = DV + 1  # 65

_prog = {}


def _build_program():
    from contextlib import ExitStack

    import concourse.mybir as mybir
    import concourse.tile as tile
    from concourse import bacc

    f32 = mybir.dt.float32
    f16 = mybir.dt.float16
    Exp = mybir.ActivationFunctionType.Exp

    nc = bacc.Bacc("TRN2", target_bir_lowering=False, debug=False, num_devices=NCORES)

    qt_d = nc.dram_tensor("qt", [NDC, P, ROWS], f16, kind="ExternalInput").ap()
    kt_d = nc.dram_tensor("kt", [S // 512, P, NDC, 512], f16, kind="ExternalInput").ap()
    vt_d = nc.dram_tensor("vt", [NTT, P, NDC, 128], f16, kind="ExternalInput").ap()
    wq_d = nc.dram_tensor("wq", [NDC, P, D], f16, kind="ExternalInput").ap()
    wk_d = nc.dram_tensor("wk", [NDC, P, D], f16, kind="ExternalInput").ap()
    wv_d = nc.dram_tensor("wv", [NDC, P, D], f16, kind="ExternalInput").ap()
    wo2_d = nc.dram_tensor("wo2", [NPAIR, P, D], f16, kind="ExternalInput").ap()
    bq_d = nc.dram_tensor("bq", [P, NPAIR], f32, kind="ExternalInput").ap()
    bk_d = nc.dram_tensor("bk", [P, NPAIR], f32, kind="ExternalInput").ap()
    bvb_d = nc.dram_tensor("bvb", [P, D], f32, kind="ExternalInput").ap()
    bob_d = nc.dram_tensor("bob", [P, D], f32, kind="ExternalInput").ap()
    out_d = nc.dram_tensor("out", [ROWS // P, P, D], f32, kind="ExternalOutput").ap()

    with tile.TileContext(nc) as tc, ExitStack() as ctx:
        weights = ctx.enter_context(tc.tile_pool(name="weights", bufs=1))
        raw = ctx.enter_context(tc.tile_pool(name="raw", bufs=1))
        acts = ctx.enter_context(tc.tile_pool(name="acts", bufs=1))
        attn_pool = ctx.enter_context(tc.tile_pool(name="attn", bufs=18))
        small = ctx.enter_context(tc.tile_pool(name="small", bufs=2))
        ps_proj = ctx.enter_context(tc.tile_pool(name="ps_proj", bufs=2, space="PSUM"))
        ps_sc = ctx.enter_context(tc.tile_pool(name="ps_sc", bufs=2, space="PSUM"))
        ps_o = ctx.enter_context(tc.tile_pool(name="ps_o", bufs=1, space="PSUM"))
        ps_rs = ctx.enter_context(tc.tile_pool(name="ps_rs", bufs=1, space="PSUM"))

        # ---------------- load phase (DMA priority order) ----------------
        wk_sb = [weights.tile([P, D], f16, tag=f"wk{c}", name=f"wk{c}") for c in range(NDC)]
        wq_sb = [weights.tile([P, D], f16, tag=f"wq{c}", name=f"wq{c}") for c in range(NDC)]
        wv_sb = [weights.tile([P, D], f16, tag=f"wv{c}", name=f"wv{c}") for c in range(NDC)]
        wo2_sb = [weights.tile([P, D], f16, tag=f"wo{p}", name=f"wo{p}") for p in range(NPAIR)]
        qt_sb = [raw.tile([P, ROWS], f16, tag=f"qt{c}", name=f"qt{c}") for c in range(NDC)]
        bq_sb = weights.tile([P, NPAIR], f32, tag="bq")
        bk_sb = weights.tile([P, NPAIR], f32, tag="bk")
        bvb_sb = weights.tile([P, D], f32, tag="bvb")
        bob_sb = weights.tile([P, D], f32, tag="bob")

        for c in range(NDC):
            nc.sync.dma_start(out=wk_sb[c], in_=wk_d[c])
        kt_slabs = []
        for g in range(S // 512):
            kt_slab = raw.tile([P, NDC, 512], f16, tag=f"kt{g}", name=f"kt_slab{g}")
            nc.sync.dma_start(out=kt_slab, in_=kt_d[g])
            kt_slabs.append(kt_slab)
        nc.sync.dma_start(out=bk_sb, in_=bk_d)
        for c in range(NDC):
            nc.sync.dma_start(out=wq_sb[c], in_=wq_d[c])
            nc.sync.dma_start(out=qt_sb[c], in_=qt_d[c])
        nc.sync.dma_start(out=bq_sb, in_=bq_d)
        for c in range(NDC):
            nc.sync.dma_start(out=wv_sb[c], in_=wv_d[c])
        vt_slabs = []
        for t in range(NTT):
            vt_slab = raw.tile([P, NDC, 128], f16, tag=f"vt{t}", name=f"vt_slab{t}")
            nc.sync.dma_start(out=vt_slab, in_=vt_d[t])
            vt_slabs.append(vt_slab)
        nc.sync.dma_start(out=bvb_sb, in_=bvb_d)
        for p in range(NPAIR):
            nc.sync.dma_start(out=wo2_sb[p], in_=wo2_d[p])
        nc.sync.dma_start(out=bob_sb, in_=bob_d)

        # ---------------- persistent compute tiles ----------------
        KT = [acts.tile([P, S], f16, tag=f"KT{p}", name=f"KT{p}") for p in range(NPAIR)]
        QT = [acts.tile([P, ROWS], f16, tag=f"QT{p}", name=f"QT{p}") for p in range(NPAIR)]
        Vt = [acts.tile([P, H, VW], f16, tag=f"Vt{t}", name=f"Vt{t}") for t in range(NTT)]
        o2Tp = [acts.tile([P, ROWS], f16, tag=f"o2T{p}", name=f"o2Tp{p}") for p in range(NPAIR)]

        # ---------------- helpers ----------------
        _ktps = {}

        def kt_chunk(p, g, c):
            """One chunk of KT[p] columns [g*512,(g+1)*512); c=0..3."""
            if c == 0:
                _ktps[(p, g)] = ps_proj.tile([P, 512], f32, tag="pp", name=f"ps_kt{p}_{g}")
            pp = _ktps[(p, g)]
            nc.tensor.matmul(
                pp, lhsT=wk_sb[c][:, p * 128 : (p + 1) * 128],
                rhs=kt_slabs[g][:, c, :],
                start=(c == 0), stop=(c == NDC - 1),
            )
            if c == NDC - 1:
                del _ktps[(p, g)]
                nc.vector.tensor_scalar_add(
                    KT[p][:, g * 512 : (g + 1) * 512], pp, bk_sb[:, p : p + 1]
                )

        def kt_group(p, g):
            for c in range(NDC):
                kt_chunk(p, g, c)

        def qt_group(p):
            pp = ps_proj.tile([P, ROWS], f32, tag="pp", name="ps_q")
            for c in range(NDC):
                nc.tensor.matmul(
                    pp, lhsT=wq_sb[c][:, p * 128 : (p + 1) * 128], rhs=qt_sb[c],
                    start=(c == 0), stop=(c == NDC - 1),
                )
            nc.vector.tensor_scalar_add(QT[p], pp, bq_sb[:, p : p + 1])

        _vps = {}

        def v_chunk(t, c):
            """One chunk of V' proj for key-tile t (call with c=0..3)."""
            if c == 0:
                _vps[t] = ps_proj.tile([P, D], f32, tag="pp", name=f"ps_v{t}")
            pp = _vps[t]
            nc.tensor.matmul(
                pp, lhsT=vt_slabs[t][:, c, :], rhs=wv_sb[c],
                start=(c == 0), stop=(c == NDC - 1),
            )
            if c == NDC - 1:
                del _vps[t]
                nc.vector.tensor_add(
                    Vt[t][:, :, 0:DV],
                    pp.rearrange("p (i v) -> p i v", i=H),
                    bvb_sb.rearrange("p (i v) -> p i v", i=H),
                )
                nc.vector.memset(Vt[t][:, :, DV : DV + 1], 1.0)

        attn_tiles = {}

        def sc_step(p, t):
            ps = ps_sc.tile([P, 2, 512], f32, tag="sc", name="ps_sc_t")
            ts = slice(t * 128, (t + 1) * 128)
            nc.tensor.matmul(
                ps[:, 0, :], lhsT=KT[p][0:64, ts], rhs=QT[p][0:64, :],
                start=True, stop=True, tile_position=(0, 0),
            )
            nc.tensor.matmul(
                ps[:, 1, :], lhsT=KT[p][64:128, ts], rhs=QT[p][64:128, :],
                start=True, stop=True, tile_position=(64, 0),
            )
            at = attn_pool.tile([P, 2, 512], f16, tag="at", name="at_t")
            nc.scalar.activation(at, ps, Exp, scale=1.0 / np.sqrt(DK))
            attn_tiles[(p, t)] = at

        pair_ps = {}

        def ov_start(p):
            pair_ps[p] = (
                ps_o.tile([VW, ROWS], f32, tag="o", name="o_psA"),
                ps_rs.tile([VW, ROWS], f32, tag="rs", name="o_psB"),
            )

        def ov_step(p, t):
            o_psA, o_psB = pair_ps[p]
            at = attn_tiles.pop((p, t))
            first, last = (t == 0), (t == NTT - 1)
            nc.tensor.matmul(
                o_psA, lhsT=Vt[t][:, 2 * p, :], rhs=at[:, 0, :],
                start=first, stop=last,
            )
            nc.tensor.matmul(
                o_psB, lhsT=Vt[t][:, 2 * p + 1, :], rhs=at[:, 1, :],
                start=first, stop=last,
            )

        def ov_finish(p):
            o_psA, o_psB = pair_ps.pop(p)
            rrA = small.tile([1, ROWS], f32, tag="rrA")
            rrB = small.tile([1, ROWS], f32, tag="rrB")
            nc.vector.reciprocal(rrA, o_psA[DV : DV + 1, :])
            nc.vector.reciprocal(rrB, o_psB[DV : DV + 1, :])
            bcA = small.tile([DV, ROWS], f32, tag="bcA")
            bcB = small.tile([DV, ROWS], f32, tag="bcB")
            nc.gpsimd.partition_broadcast(bcA, rrA, channels=DV)
            nc.gpsimd.partition_broadcast(bcB, rrB, channels=DV)
            nc.vector.tensor_mul(o2Tp[p][0:DV, :], o_psA[0:DV, :], bcA)
            nc.vector.tensor_mul(o2Tp[p][DV:P, :], o_psB[0:DV, :], bcB)

        # ---------------- schedule ----------------
        # lead-in: KT0 + KT1 + QT on the PE while inputs stream in
        for g in range(S // 512):
            kt_group(0, g)
        for g in range(S // 512):
            kt_group(1, g)
        for p in range(NPAIR):
            qt_group(p)

        # window 0: scores(0) + V' proj interleaved (one tile's 4 chunks/step)
        for t in range(NTT):
            sc_step(0, t)
            for c in range(NDC):
                v_chunk(t, c)

        # windows 1..3: scores(p) + ov(p-1) + KT[p+1] chunks interleaved
        for p in range(1, NPAIR):
            ov_start(p - 1)
            for t in range(NTT):
                sc_step(p, t)
                ov_step(p - 1, t)
                if p + 1 < NPAIR:
                    kt_chunk(p + 1, t // 4, t % 4)
            ov_finish(p - 1)

        ov_start(NPAIR - 1)
        for t in range(NTT):
            ov_step(NPAIR - 1, t)
        ov_finish(NPAIR - 1)

        # output projection (pair-stacked, C=128)
        for st in range(ROWS // P):
            pp = ps_proj.tile([P, D], f32, tag="pp", name="ps_out")
            for p in range(NPAIR):
                nc.tensor.matmul(
                    pp, lhsT=o2Tp[p][:, st * 128 : (st + 1) * 128], rhs=wo2_sb[p],
                    start=(p == 0), stop=(p == NPAIR - 1),
                )
            ot = small.tile([P, D], f32, tag="ot")
            nc.vector.tensor_add(ot, pp, bob_sb)
            nc.sync.dma_start(out=out_d[st], in_=ot)

    nc.compile()
    return nc


def _get_program():
    if "p" not in _prog:
        _prog["p"] = _build_program()
    return _prog["p"]


def _stage_inputs(queries, keys, values, wq, bq, wk, bk, wv, bv, wo, bo):
    """Host staging: transpose activations to [D, S], per-core shards."""
    h = np.float16
    qT = queries.transpose(0, 2, 1).astype(h)
    kT = keys.transpose(0, 2, 1).astype(h)
    vT = values.transpose(0, 2, 1).astype(h)

    def chunk(m):
        return np.ascontiguousarray(m.reshape(NDC, P, m.shape[1]))

    wq_m = chunk(np.concatenate([wq[i] for i in range(H)], axis=1)).astype(h)
    wk_m = chunk(np.concatenate([wk[i] for i in range(H)], axis=1)).astype(h)
    wv_m = chunk(np.concatenate([wv[i] for i in range(H)], axis=1)).astype(h)
    wo2_m = np.ascontiguousarray(wo.reshape(NPAIR, P, D)).astype(h)
    bq_m = np.ascontiguousarray(bq.reshape(NPAIR, P).T.astype(np.float32))
    bk_m = np.ascontiguousarray(bk.reshape(NPAIR, P).T.astype(np.float32))
    bvb = np.broadcast_to(bv.reshape(1, D), (P, D)).astype(np.float32).copy()
    bob = np.broadcast_to(bo.reshape(1, D), (P, D)).astype(np.float32).copy()

    kt_b = [
        np.ascontiguousarray(kT[b].reshape(NDC, P, S // 512, 512).transpose(2, 1, 0, 3))
        for b in range(B)
    ]
    vt_b = [
        np.ascontiguousarray(vT[b].reshape(NDC, P, NTT, 128).transpose(2, 1, 0, 3))
        for b in range(B)
    ]
    in_maps = []
    for c in range(NCORES):
        b, r = c // 4, c % 4
        qt_c = np.ascontiguousarray(
            qT[b][:, r * ROWS : (r + 1) * ROWS].reshape(NDC, P, ROWS)
        )
        in_maps.append(
            {
                "qt": qt_c, "kt": kt_b[b], "vt": vt_b[b],
                "wq": wq_m, "wk": wk_m, "wv": wv_m, "wo2": wo2_m,
                "bq": bq_m, "bk": bk_m, "bvb": bvb, "bob": bob,
            }
        )
    return in_maps


def run(trace=False, **inputs):
    from concourse.bass_utils import run_bass_kernel_spmd

    nc = _get_program()
    in_maps = _stage_inputs(**inputs)
    res = run_bass_kernel_spmd(nc, in_maps, core_ids=list(range(NCORES)), trace=trace)
    out = np.empty((B, S, D), np.float32)
    for c in range(NCORES):
        b, r = c // 4, c % 4
        out[b, r * ROWS : (r + 1) * ROWS, :] = res.results[c]["out"].reshape(ROWS, D)
    return out, res


def kernel(**inputs):
    out, _ = run(trace=False, **inputs)
    return out


# revision 9
# speedup vs baseline: 1.0131x; 1.0131x over previous
"""Multi-head attention kernel for 8 Trainium2 NeuronCores.

Problem: B=2, S=2048, H=8, DK=DV=64, D=512 (nn_MultiHeadAttention).

Sharding: core c owns batch b=c//4 and query rows [512*r, 512*r+512),
r = c%4. No collectives: every core computes the K/V projections for all
8 heads locally (K/V proj is cheap at full PE clock; the 4-way AllGather
it would replace costs ~57us and stalls the PE).

Math note: the key bias bk drops out of softmax entirely (it adds a
per-query-row constant to every score), so KT is projected bias-free.

Per-core device kernel (heads processed as 4 pairs of 2):
  KT[p]  = wk[:,pair p].T @ kT           [128, 2048] fp16 (no bias)
  QT[p]  = wq[:,pair p].T @ qT + bq      [128, 512]  fp16
  V'[t]  = vT[t].T @ wv + bv | 1         [128, 8, 65] fp16 (ones col
           makes the ov matmul emit the softmax denominator in row 64)
  scores pair-step: two C=64 matmuls at tile_position (0,0)/(64,0) run
           concurrently on the PE array halves -> [128, 2, 512] psum
  attnT  = exp(scores/8) on ScalarE, fp16, no max-subtract
  o65   += V'[t,h].T @ attnT[h], accumulated over t (rows 0:64 = head
           output, row 64 = denominator)
  o2T[p] = o65[0:64] * (1/o65[64]) via DVE recip + GPSIMD
           partition_broadcast + DVE mul, packed per pair [128, 512]
  out    = sum_p o2T[p].T-chunks @ wo2[p] + bo   (C=128 pair-stacked;
           pairs 0..2 pre-accumulate into freed scores-psum during the
           final ov window, pair 3 closes the groups)

Projection matmuls are interleaved into the attention windows so the PE
never idles (DVFS keeps the 2.4GHz clock only while the PE is dense).
"""

import numpy as np

B, S, H, DK, DV = 2, 2048, 8, 64, 64
D = H * DV  # 512
NCORES = 8
ROWS = (B * S) // NCORES  # 512 query rows per core
NPAIR = H // 2  # 4 head pairs
NTT = S // 128  # 16 key/value tiles
NDC = D // 128  # 4 contraction chunks
P = 128
VW = DV + 1  # 65

_prog = {}


def _build_program():
    from contextlib import ExitStack

    import concourse.mybir as mybir
    import concourse.tile as tile
    from concourse import bacc

    f32 = mybir.dt.float32
    f16 = mybir.dt.float16
    Exp = mybir.ActivationFunctionType.Exp

    nc = bacc.Bacc("TRN2", target_bir_lowering=False, debug=False, num_devices=NCORES)

    qt_d = nc.dram_tensor("qt", [P, NDC, ROWS], f16, kind="ExternalInput").ap()
    kt_d = nc.dram_tensor("kt", [S // 512, P, NDC, 512], f16, kind="ExternalInput").ap()
    vt_d = nc.dram_tensor("vt", [NTT // 4, P, NDC, 4, 128], f16, kind="ExternalInput").ap()
    wq_d = nc.dram_tensor("wq", [P, NDC, D], f16, kind="ExternalInput").ap()
    wk_d = nc.dram_tensor("wk", [P, NDC, D], f16, kind="ExternalInput").ap()
    wv_d = nc.dram_tensor("wv", [P, NDC, D], f16, kind="ExternalInput").ap()
    wo2_d = nc.dram_tensor("wo2", [NPAIR, P, D], f16, kind="ExternalInput").ap()
    bq_d = nc.dram_tensor("bq", [P, NPAIR], f32, kind="ExternalInput").ap()
    bvb_d = nc.dram_tensor("bvb", [P, D], f32, kind="ExternalInput").ap()
    bob_d = nc.dram_tensor("bob", [P, D], f32, kind="ExternalInput").ap()
    out_d = nc.dram_tensor("out", [ROWS // P, P, D], f32, kind="ExternalOutput").ap()

    with tile.TileContext(nc) as tc, ExitStack() as ctx:
        weights = ctx.enter_context(tc.tile_pool(name="weights", bufs=1))
        raw = ctx.enter_context(tc.tile_pool(name="raw", bufs=1))
        acts = ctx.enter_context(tc.tile_pool(name="acts", bufs=1))
        attn_pool = ctx.enter_context(tc.tile_pool(name="attn", bufs=18))
        small = ctx.enter_context(tc.tile_pool(name="small", bufs=2))
        ps_proj = ctx.enter_context(tc.tile_pool(name="ps_proj", bufs=2, space="PSUM"))
        ps_sc = ctx.enter_context(tc.tile_pool(name="ps_sc", bufs=2, space="PSUM"))
        ps_o = ctx.enter_context(tc.tile_pool(name="ps_o", bufs=1, space="PSUM"))
        ps_rs = ctx.enter_context(tc.tile_pool(name="ps_rs", bufs=1, space="PSUM"))

        # ---------------- load phase (DMA priority order) ----------------
        wk_sb = weights.tile([P, NDC, D], f16, tag="wk")
        wq_sb = weights.tile([P, NDC, D], f16, tag="wq")
        wv_sb = weights.tile([P, NDC, D], f16, tag="wv")
        wo2_sb = [weights.tile([P, D], f16, tag=f"wo{p}", name=f"wo{p}") for p in range(NPAIR)]
        qt_sb = raw.tile([P, NDC, ROWS], f16, tag="qt")
        bq_sb = weights.tile([P, NPAIR], f32, tag="bq")
        bvb_sb = weights.tile([P, D], f32, tag="bvb")
        bob_sb = weights.tile([P, D], f32, tag="bob")

        nc.sync.dma_start(out=wk_sb, in_=wk_d)
        kt_slabs = []
        for g in range(S // 512):
            kt_slab = raw.tile([P, NDC, 512], f16, tag=f"kt{g}", name=f"kt_slab{g}")
            nc.sync.dma_start(out=kt_slab, in_=kt_d[g])
            kt_slabs.append(kt_slab)
        nc.sync.dma_start(out=wq_sb, in_=wq_d)
        nc.sync.dma_start(out=qt_sb, in_=qt_d)
        nc.sync.dma_start(out=bq_sb, in_=bq_d)
        nc.sync.dma_start(out=wv_sb, in_=wv_d)
        vt_slabs = []
        for gv in range(NTT // 4):
            vt_slab = raw.tile([P, NDC, 4, 128], f16, tag=f"vt{gv}", name=f"vt_slab{gv}")
            nc.sync.dma_start(out=vt_slab, in_=vt_d[gv])
            vt_slabs.append(vt_slab)
        nc.sync.dma_start(out=bvb_sb, in_=bvb_d)
        for p in range(NPAIR):
            nc.sync.dma_start(out=wo2_sb[p], in_=wo2_d[p])
        nc.sync.dma_start(out=bob_sb, in_=bob_d)

        # ---------------- persistent compute tiles ----------------
        KT = [acts.tile([P, S], f16, tag=f"KT{p}", name=f"KT{p}") for p in range(NPAIR)]
        QT = [acts.tile([P, ROWS], f16, tag=f"QT{p}", name=f"QT{p}") for p in range(NPAIR)]
        Vt = [acts.tile([P, H, VW], f16, tag=f"Vt{t}", name=f"Vt{t}") for t in range(NTT)]
        o2Tp = [acts.tile([P, ROWS], f16, tag=f"o2T{p}", name=f"o2Tp{p}") for p in range(NPAIR)]

        # ---------------- helpers ----------------
        _ktps = {}

        def kt_chunk(p, g, c):
            """One chunk of KT[p] columns [g*512,(g+1)*512); c=0..3."""
            if c == 0:
                _ktps[(p, g)] = ps_proj.tile([P, 512], f32, tag="pp", name=f"ps_kt{p}_{g}")
            pp = _ktps[(p, g)]
            nc.tensor.matmul(
                pp, lhsT=wk_sb[:, c, p * 128 : (p + 1) * 128],
                rhs=kt_slabs[g][:, c, :],
                start=(c == 0), stop=(c == NDC - 1),
            )
            if c == NDC - 1:
                del _ktps[(p, g)]
                nc.vector.tensor_copy(KT[p][:, g * 512 : (g + 1) * 512], pp)

        def kt_group(p, g):
            for c in range(NDC):
                kt_chunk(p, g, c)

        def qt_group(p):
            pp = ps_proj.tile([P, ROWS], f32, tag="pp", name="ps_q")
            for c in range(NDC):
                nc.tensor.matmul(
                    pp, lhsT=wq_sb[:, c, p * 128 : (p + 1) * 128], rhs=qt_sb[:, c, :],
                    start=(c == 0), stop=(c == NDC - 1),
                )
            nc.vector.tensor_scalar_add(QT[p], pp, bq_sb[:, p : p + 1])

        _vps = {}

        def v_chunk(t, c):
            """One chunk of V' proj for key-tile t (c=0..3)."""
            if c == 0:
                _vps[t] = ps_proj.tile([P, D], f32, tag="pp", name=f"ps_v{t}")
            pp = _vps[t]
            nc.tensor.matmul(
                pp, lhsT=vt_slabs[t // 4][:, c, t % 4, :], rhs=wv_sb[:, c, :],
                start=(c == 0), stop=(c == NDC - 1),
            )
            if c == NDC - 1:
                del _vps[t]
                nc.vector.tensor_add(
                    Vt[t][:, :, 0:DV],
                    pp.rearrange("p (i v) -> p i v", i=H),
                    bvb_sb.rearrange("p (i v) -> p i v", i=H),
                )
                nc.vector.memset(Vt[t][:, :, DV : DV + 1], 1.0)

        attn_tiles = {}

        def sc_step(p, t):
            ps = ps_sc.tile([P, 2, 512], f32, tag="sc", name="ps_sc_t")
            ts = slice(t * 128, (t + 1) * 128)
            nc.tensor.matmul(
                ps[:, 0, :], lhsT=KT[p][0:64, ts], rhs=QT[p][0:64, :],
                start=True, stop=True, tile_position=(0, 0),
            )
            nc.tensor.matmul(
                ps[:, 1, :], lhsT=KT[p][64:128, ts], rhs=QT[p][64:128, :],
                start=True, stop=True, tile_position=(64, 0),
            )
            at = attn_pool.tile([P, 2, 512], f16, tag="at", name="at_t")
            nc.scalar.activation(at, ps, Exp, scale=1.0 / np.sqrt(DK))
            attn_tiles[(p, t)] = at

        pair_ps = {}

        def ov_start(p):
            pair_ps[p] = (
                ps_o.tile([VW, ROWS], f32, tag="o", name="o_psA"),
                ps_rs.tile([VW, ROWS], f32, tag="rs", name="o_psB"),
            )

        def ov_step(p, t):
            o_psA, o_psB = pair_ps[p]
            at = attn_tiles.pop((p, t))
            first, last = (t == 0), (t == NTT - 1)
            nc.tensor.matmul(
                o_psA, lhsT=Vt[t][:, 2 * p, :], rhs=at[:, 0, :],
                start=first, stop=last,
            )
            nc.tensor.matmul(
                o_psB, lhsT=Vt[t][:, 2 * p + 1, :], rhs=at[:, 1, :],
                start=first, stop=last,
            )

        def ov_finish(p):
            """Free the ov psum banks fast (copies + recips are the only
            readers), then normalize off the critical path."""
            o_psA, o_psB = pair_ps.pop(p)
            rrA = small.tile([1, ROWS], f32, tag="rrA")
            rrB = small.tile([1, ROWS], f32, tag="rrB")
            opkA = small.tile([DV, ROWS], f32, tag="opkA")
            opkB = small.tile([DV, ROWS], f32, tag="opkB")
            nc.vector.reciprocal(rrA, o_psA[DV : DV + 1, :])
            nc.vector.tensor_copy(opkA, o_psA[0:DV, :])
            nc.vector.reciprocal(rrB, o_psB[DV : DV + 1, :])
            nc.vector.tensor_copy(opkB, o_psB[0:DV, :])
            bcA = small.tile([DV, ROWS], f32, tag="bcA")
            bcB = small.tile([DV, ROWS], f32, tag="bcB")
            nc.gpsimd.partition_broadcast(bcA, rrA, channels=DV)
            nc.gpsimd.partition_broadcast(bcB, rrB, channels=DV)
            nc.vector.tensor_mul(o2Tp[p][0:DV, :], opkA, bcA)
            nc.vector.tensor_mul(o2Tp[p][DV:P, :], opkB, bcB)

        # ---------------- schedule ----------------
        # lead-in: KT0 + QT on the PE while inputs stream in
        for g in range(S // 512):
            kt_group(0, g)
        for p in range(NPAIR):
            qt_group(p)

        # window 0: scores(0) + V' proj interleaved (one tile's 4 chunks/step)
        for t in range(NTT):
            sc_step(0, t)
            for c in range(NDC):
                v_chunk(t, c)

        # KT1 block (Act drains the exp(0) backlog meanwhile)
        for g in range(S // 512):
            kt_group(1, g)

        # windows 1..3: scores(p) + ov(p-1) + KT[p+1] chunks interleaved
        for p in range(1, NPAIR):
            ov_start(p - 1)
            for t in range(NTT):
                sc_step(p, t)
                ov_step(p - 1, t)
                if p + 1 < NPAIR:
                    kt_chunk(p + 1, t // 4, t % 4)
            ov_finish(p - 1)

        # final ov window: ov(3) + out-projection partials for pairs 0..2
        # (scores psum is free now; [P,2,512]-shaped tiles hold 2 st each)
        out_ps = [
            ps_sc.tile([P, 2, 512], f32, tag="sc", name="out_psA"),
            ps_sc.tile([P, 2, 512], f32, tag="sc", name="out_psB"),
        ]
        ov_start(NPAIR - 1)
        op_jobs = [(st, p) for st in range(ROWS // P) for p in range(NPAIR - 1)]
        for t in range(NTT):
            ov_step(NPAIR - 1, t)
            if t < len(op_jobs):
                st, p = op_jobs[t]
                nc.tensor.matmul(
                    out_ps[st // 2][:, st % 2, :],
                    lhsT=o2Tp[p][:, st * 128 : (st + 1) * 128], rhs=wo2_sb[p],
                    start=(p == 0), stop=False,
                )
        ov_finish(NPAIR - 1)

        # close the out-projection groups with pair 3, add bias, store
        for st in range(ROWS // P):
            nc.tensor.matmul(
                out_ps[st // 2][:, st % 2, :],
                lhsT=o2Tp[NPAIR - 1][:, st * 128 : (st + 1) * 128],
                rhs=wo2_sb[NPAIR - 1],
                start=False, stop=True,
            )
            ot = small.tile([P, D], f32, tag=f"ot{st % 2}", name=f"ot{st}")
            nc.vector.tensor_add(ot, out_ps[st // 2][:, st % 2, :], bob_sb)
            nc.sync.dma_start(out=out_d[st], in_=ot)

    nc.compile()
    return nc


def _get_program():
    if "p" not in _prog:
        _prog["p"] = _build_program()
    return _prog["p"]


def _stage_inputs(queries, keys, values, wq, bq, wk, bk, wv, bv, wo, bo):
    """Host staging: transpose activations to [D, S], per-core shards.
    bk is accepted and ignored (softmax-invariant)."""
    h = np.float16
    qT = queries.transpose(0, 2, 1).astype(h)
    kT = keys.transpose(0, 2, 1).astype(h)
    vT = values.transpose(0, 2, 1).astype(h)

    def wstage(w):
        # [H, D, 64] -> [P, NDC, D]: out[p, c, j] = W[c*128+p, j] (concat heads)
        wf = np.concatenate([w[i] for i in range(H)], axis=1)  # [D, D]
        return np.ascontiguousarray(wf.reshape(NDC, P, D).transpose(1, 0, 2)).astype(h)

    wq_m = wstage(wq)
    wk_m = wstage(wk)
    wv_m = wstage(wv)
    wo2_m = np.ascontiguousarray(wo.reshape(NPAIR, P, D)).astype(h)
    bq_m = np.ascontiguousarray(bq.reshape(NPAIR, P).T.astype(np.float32))
    bvb = np.broadcast_to(bv.reshape(1, D), (P, D)).astype(np.float32).copy()
    bob = np.broadcast_to(bo.reshape(1, D), (P, D)).astype(np.float32).copy()

    kt_b = [
        np.ascontiguousarray(kT[b].reshape(NDC, P, S // 512, 512).transpose(2, 1, 0, 3))
        for b in range(B)
    ]
    vt_b = [
        np.ascontiguousarray(
            vT[b].reshape(NDC, P, NTT // 4, 4, 128).transpose(2, 1, 0, 3, 4)
        )
        for b in range(B)
    ]
    in_maps = []
    for c in range(NCORES):
        b, r = c // 4, c % 4
        qt_c = np.ascontiguousarray(
            qT[b][:, r * ROWS : (r + 1) * ROWS].reshape(NDC, P, ROWS).transpose(1, 0, 2)
        )
        in_maps.append(
            {
                "qt": qt_c, "kt": kt_b[b], "vt": vt_b[b],
                "wq": wq_m, "wk": wk_m, "wv": wv_m, "wo2": wo2_m,
                "bq": bq_m, "bvb": bvb, "bob": bob,
            }
        )
    return in_maps


def run(trace=False, **inputs):
    from concourse.bass_utils import run_bass_kernel_spmd

    nc = _get_program()
    in_maps = _stage_inputs(**inputs)
    res = run_bass_kernel_spmd(nc, in_maps, core_ids=list(range(NCORES)), trace=trace)
    out = np.empty((B, S, D), np.float32)
    for c in range(NCORES):
        b, r = c // 4, c % 4
        out[b, r * ROWS : (r + 1) * ROWS, :] = res.results[c]["out"].reshape(ROWS, D)
    return out, res


def kernel(**inputs):
    out, _ = run(trace=False, **inputs)
    return out


# revision 14
# speedup vs baseline: 1.0273x; 1.0140x over previous
"""Multi-head attention kernel for 8 Trainium2 NeuronCores.

Problem: B=2, S=2048, H=8, DK=DV=64, D=512 (nn_MultiHeadAttention).

Sharding: core c owns batch b=c//4 and query rows [512*r, 512*r+512),
r = c%4. No collectives: every core computes the K/V projections for all
8 heads locally (K/V proj is cheap at full PE clock; the 4-way AllGather
it would replace costs ~57us and stalls the PE).

Math note: the key bias bk drops out of softmax entirely (it adds a
per-query-row constant to every score), so KT is projected bias-free.

Per-core device kernel (heads processed as 4 pairs of 2):
  KT[p]  = wk[:,pair p].T @ kT           [128, 2048] fp16 (no bias)
  QT[p]  = wq[:,pair p].T @ qT + bq      [128, 512]  fp16
  V'[t]  = vT[t].T @ wv + bv | 1         [128, 8, 65] fp16 (ones col
           makes the ov matmul emit the softmax denominator in row 64)
  scores pair-step: two C=64 matmuls at tile_position (0,0)/(64,0) run
           concurrently on the PE array halves -> [128, 2, 512] psum
  attnT  = exp(scores/8) on ScalarE, fp16, no max-subtract
  o65   += V'[t,h].T @ attnT[h], accumulated over t (rows 0:64 = head
           output, row 64 = denominator)
  o2T[p] = o65[0:64] * (1/o65[64]) via DVE recip + GPSIMD
           partition_broadcast + DVE mul, packed per pair [128, 512]
  out    = sum_p o2T[p].T-chunks @ wo2[p] + bo   (C=128 pair-stacked;
           pairs 0..2 pre-accumulate into freed scores-psum during the
           final ov window, pair 3 closes the groups)

Projection matmuls are interleaved into the attention windows so the PE
never idles (DVFS keeps the 2.4GHz clock only while the PE is dense).
"""

import numpy as np

B, S, H, DK, DV = 2, 2048, 8, 64, 64
D = H * DV  # 512
NCORES = 8
ROWS = (B * S) // NCORES  # 512 query rows per core
NPAIR = H // 2  # 4 head pairs
NTT = S // 128  # 16 key/value tiles
NDC = D // 128  # 4 contraction chunks
P = 128
VW = DV + 1  # 65

_prog = {}


def _build_program():
    from contextlib import ExitStack

    import concourse.mybir as mybir
    import concourse.tile as tile
    from concourse import bacc

    f32 = mybir.dt.float32
    f16 = mybir.dt.float16
    Exp = mybir.ActivationFunctionType.Exp

    nc = bacc.Bacc("TRN2", target_bir_lowering=False, debug=False, num_devices=NCORES)

    qt_d = nc.dram_tensor("qt", [P, NDC, ROWS], f16, kind="ExternalInput").ap()
    kt_d = nc.dram_tensor("kt", [S // 512, P, NDC, 512], f16, kind="ExternalInput").ap()
    vt_d = nc.dram_tensor("vt", [NTT // 4, P, NDC, 4, 128], f16, kind="ExternalInput").ap()
    wq_d = nc.dram_tensor("wq", [P, NDC, D], f16, kind="ExternalInput").ap()
    wk_d = nc.dram_tensor("wk", [P, NDC, D], f16, kind="ExternalInput").ap()
    wv_d = nc.dram_tensor("wv", [P, NDC, D], f16, kind="ExternalInput").ap()
    wo2_d = nc.dram_tensor("wo2", [NPAIR, P, D], f16, kind="ExternalInput").ap()
    bq_d = nc.dram_tensor("bq", [P, NPAIR], f32, kind="ExternalInput").ap()
    bv1_d = nc.dram_tensor("bv1", [1, D], f32, kind="ExternalInput").ap()
    bo1_d = nc.dram_tensor("bo1", [1, D], f32, kind="ExternalInput").ap()
    out_d = nc.dram_tensor("out", [ROWS // P, P, D], f32, kind="ExternalOutput").ap()

    with tile.TileContext(nc) as tc, ExitStack() as ctx:
        weights = ctx.enter_context(tc.tile_pool(name="weights", bufs=1))
        raw = ctx.enter_context(tc.tile_pool(name="raw", bufs=1))
        acts = ctx.enter_context(tc.tile_pool(name="acts", bufs=1))
        attn_pool = ctx.enter_context(tc.tile_pool(name="attn", bufs=22))
        small = ctx.enter_context(tc.tile_pool(name="small", bufs=2))
        ps_proj = ctx.enter_context(tc.tile_pool(name="ps_proj", bufs=2, space="PSUM"))
        ps_sc = ctx.enter_context(tc.tile_pool(name="ps_sc", bufs=2, space="PSUM"))
        ps_o = ctx.enter_context(tc.tile_pool(name="ps_o", bufs=1, space="PSUM"))
        ps_rs = ctx.enter_context(tc.tile_pool(name="ps_rs", bufs=1, space="PSUM"))

        # ---------------- load phase (DMA priority order) ----------------
        wk_sb = weights.tile([P, NDC, D], f16, tag="wk")
        wq_sb = weights.tile([P, NDC, D], f16, tag="wq")
        wv_sb = weights.tile([P, NDC, D], f16, tag="wv")
        wo2_sb = [weights.tile([P, D], f16, tag=f"wo{p}", name=f"wo{p}") for p in range(NPAIR)]
        qt_sb = raw.tile([P, NDC, ROWS], f16, tag="qt")
        bq_sb = weights.tile([P, NPAIR], f32, tag="bq")
        bvb_sb = weights.tile([P, D], f32, tag="bvb")
        bob_sb = weights.tile([P, D], f32, tag="bob")

        bv1_sb = weights.tile([1, D], f32, tag="bv1")
        bo1_sb = weights.tile([1, D], f32, tag="bo1")

        kt_slabs = [
            raw.tile([P, NDC, 512], f16, tag=f"kt{g}", name=f"kt_slab{g}")
            for g in range(S // 512)
        ]
        vt_slabs = [
            raw.tile([P, NDC, 4, 128], f16, tag=f"vt{gv}", name=f"vt_slab{gv}")
            for gv in range(NTT // 4)
        ]
        nc.sync.dma_start(out=wk_sb, in_=wk_d)
        nc.sync.dma_start(out=kt_slabs[0], in_=kt_d[0])
        nc.sync.dma_start(out=wq_sb, in_=wq_d)
        nc.sync.dma_start(out=qt_sb, in_=qt_d)
        nc.sync.dma_start(out=bq_sb, in_=bq_d)
        nc.sync.dma_start(out=bv1_sb, in_=bv1_d)
        nc.sync.dma_start(out=bo1_sb, in_=bo1_d)
        # broadcast the per-column biases to all partitions on-device
        nc.gpsimd.partition_broadcast(bvb_sb, bv1_sb, channels=P)
        nc.gpsimd.partition_broadcast(bob_sb, bo1_sb, channels=P)
        nc.sync.dma_start(out=kt_slabs[1], in_=kt_d[1])
        nc.sync.dma_start(out=wv_sb, in_=wv_d)
        nc.sync.dma_start(out=vt_slabs[0], in_=vt_d[0])
        nc.sync.dma_start(out=kt_slabs[2], in_=kt_d[2])
        nc.sync.dma_start(out=vt_slabs[1], in_=vt_d[1])
        nc.sync.dma_start(out=kt_slabs[3], in_=kt_d[3])
        nc.sync.dma_start(out=vt_slabs[2], in_=vt_d[2])
        nc.sync.dma_start(out=vt_slabs[3], in_=vt_d[3])
        for p in range(NPAIR):
            nc.sync.dma_start(out=wo2_sb[p], in_=wo2_d[p])

        # ---------------- persistent compute tiles ----------------
        KT = [acts.tile([P, S], f16, tag=f"KT{p}", name=f"KT{p}") for p in range(NPAIR)]
        QT = [acts.tile([P, ROWS], f16, tag=f"QT{p}", name=f"QT{p}") for p in range(NPAIR)]
        Vt = [acts.tile([P, H, VW], f16, tag=f"Vt{t}", name=f"Vt{t}") for t in range(NTT)]
        o2Tp = [acts.tile([P, ROWS], f16, tag=f"o2T{p}", name=f"o2Tp{p}") for p in range(NPAIR)]

        # ---------------- helpers ----------------
        _ktps = {}

        def kt_chunk(p, g, c):
            """One chunk of KT[p] columns [g*512,(g+1)*512); c=0..3."""
            if c == 0:
                _ktps[(p, g)] = ps_proj.tile([P, 512], f32, tag="pp", name=f"ps_kt{p}_{g}")
            pp = _ktps[(p, g)]
            nc.tensor.matmul(
                pp, lhsT=wk_sb[:, c, p * 128 : (p + 1) * 128],
                rhs=kt_slabs[g][:, c, :],
                start=(c == 0), stop=(c == NDC - 1),
            )
            if c == NDC - 1:
                del _ktps[(p, g)]
                nc.vector.tensor_copy(KT[p][:, g * 512 : (g + 1) * 512], pp)

        def kt_group(p, g):
            for c in range(NDC):
                kt_chunk(p, g, c)

        def qt_group(p):
            pp = ps_proj.tile([P, ROWS], f32, tag="pp", name="ps_q")
            for c in range(NDC):
                nc.tensor.matmul(
                    pp, lhsT=wq_sb[:, c, p * 128 : (p + 1) * 128], rhs=qt_sb[:, c, :],
                    start=(c == 0), stop=(c == NDC - 1),
                )
            nc.vector.tensor_scalar_add(QT[p], pp, bq_sb[:, p : p + 1])

        _vps = {}

        def v_chunk(t, c):
            """One chunk of V' proj for key-tile t (c=0..3)."""
            if c == 0:
                _vps[t] = ps_proj.tile([P, D], f32, tag="pp", name=f"ps_v{t}")
            pp = _vps[t]
            nc.tensor.matmul(
                pp, lhsT=vt_slabs[t // 4][:, c, t % 4, :], rhs=wv_sb[:, c, :],
                start=(c == 0), stop=(c == NDC - 1),
            )
            if c == NDC - 1:
                del _vps[t]
                nc.vector.tensor_add(
                    Vt[t][:, :, 0:DV],
                    pp.rearrange("p (i v) -> p i v", i=H),
                    bvb_sb.rearrange("p (i v) -> p i v", i=H),
                )
                nc.vector.memset(Vt[t][:, :, DV : DV + 1], 1.0)

        attn_tiles = {}

        def sc_step(p, t):
            ps = ps_sc.tile([P, 2, 512], f32, tag="sc", name="ps_sc_t")
            ts = slice(t * 128, (t + 1) * 128)
            nc.tensor.matmul(
                ps[:, 0, :], lhsT=KT[p][0:64, ts], rhs=QT[p][0:64, :],
                start=True, stop=True, tile_position=(0, 0),
            )
            nc.tensor.matmul(
                ps[:, 1, :], lhsT=KT[p][64:128, ts], rhs=QT[p][64:128, :],
                start=True, stop=True, tile_position=(64, 0),
            )
            at = attn_pool.tile([P, 2, 512], f16, tag="at", name="at_t")
            nc.scalar.activation(at, ps, Exp, scale=1.0 / np.sqrt(DK))
            attn_tiles[(p, t)] = at

        pair_ps = {}

        def ov_start(p):
            pair_ps[p] = (
                ps_o.tile([VW, ROWS], f32, tag="o", name="o_psA"),
                ps_rs.tile([VW, ROWS], f32, tag="rs", name="o_psB"),
            )

        def ov_step(p, t):
            o_psA, o_psB = pair_ps[p]
            at = attn_tiles.pop((p, t))
            first, last = (t == 0), (t == NTT - 1)
            nc.tensor.matmul(
                o_psA, lhsT=Vt[t][:, 2 * p, :], rhs=at[:, 0, :],
                start=first, stop=last,
            )
            nc.tensor.matmul(
                o_psB, lhsT=Vt[t][:, 2 * p + 1, :], rhs=at[:, 1, :],
                start=first, stop=last,
            )

        def ov_finish(p):
            """Free the ov psum banks fast (copies + recips are the only
            readers), then normalize off the critical path."""
            o_psA, o_psB = pair_ps.pop(p)
            rrA = small.tile([1, ROWS], f32, tag="rrA")
            rrB = small.tile([1, ROWS], f32, tag="rrB")
            opkA = small.tile([DV, ROWS], f32, tag="opkA")
            opkB = small.tile([DV, ROWS], f32, tag="opkB")
            nc.vector.reciprocal(rrA, o_psA[DV : DV + 1, :])
            nc.vector.tensor_copy(opkA, o_psA[0:DV, :])
            nc.vector.reciprocal(rrB, o_psB[DV : DV + 1, :])
            nc.vector.tensor_copy(opkB, o_psB[0:DV, :])
            bcA = small.tile([DV, ROWS], f32, tag="bcA")
            bcB = small.tile([DV, ROWS], f32, tag="bcB")
            nc.gpsimd.partition_broadcast(bcA, rrA, channels=DV)
            nc.gpsimd.partition_broadcast(bcB, rrB, channels=DV)
            nc.vector.tensor_mul(o2Tp[p][0:DV, :], opkA, bcA)
            nc.vector.tensor_mul(o2Tp[p][DV:P, :], opkB, bcB)

        # ---------------- out-projection helpers ----------------
        _oset = {}

        def op_tile(st):
            if st < 2:
                if st not in _oset:
                    _oset[st] = ps_proj.tile([P, 512], f32, tag="pp", name=f"out_ps{st}")
                return _oset[st]
            if "hi" not in _oset:
                _oset["hi"] = ps_sc.tile([P, 2, 512], f32, tag="sc", name="out_ps_hi")
            return _oset["hi"][:, st - 2, :]

        def op_job(st, p, stop=False):
            nc.tensor.matmul(
                op_tile(st), lhsT=o2Tp[p][:, st * 128 : (st + 1) * 128],
                rhs=wo2_sb[p], start=(p == 0), stop=stop,
            )

        def ov_job(j):
            p, t = j // NTT, j % NTT
            if t == 0:
                ov_start(p)
            ov_step(p, t)
            if t == NTT - 1:
                ov_finish(p)

        # ---------------- schedule ----------------
        # One step per scores pair-step; the ov stream trails by OVLAG=18
        # steps (one window + handoff margin), KT chunks and V' chunks
        # stream through the step slots, out-projection partials fill the
        # last steps / the tail once their pair's norm is complete.
        OVLAG = 18
        NSTEP = NPAIR * NTT
        kt_jobs = [
            (p, g, c)
            for p in range(NPAIR)
            for g in range(S // 512)
            for c in range(NDC)
            if not (p == 0 and g == 0)
        ]
        # in-stream partials: only pairs 0/1 (their norms land in-stream)
        OP_AT = {60: (0, 0), 61: (0, 1), 62: (1, 0), 63: (1, 1)}
        op_tail = [(0, 2), (1, 2), (2, 0), (2, 1), (2, 2), (3, 0), (3, 1), (3, 2)]

        # lead-in: first KT0 group + QT
        kt_group(0, 0)
        for p in range(NPAIR):
            qt_group(p)

        for s in range(NSTEP):
            sc_step(s // NTT, s % NTT)
            if s >= OVLAG:
                ov_job(s - OVLAG)
            if s < len(kt_jobs):
                kt_chunk(*kt_jobs[s])
            if 2 <= s < 2 + NTT:
                for c in range(NDC):
                    v_chunk(s - 2, c)
            if s in OP_AT:
                op_job(*OP_AT[s])

        # tail: remaining ov jobs; partials resume after ov_finish(2)
        for j in range(NSTEP - OVLAG, NPAIR * NTT):
            ov_job(j)
            if j >= 3 * NTT and (j % 2 == 0) and op_tail:
                op_job(*op_tail.pop(0))
        while op_tail:
            op_job(*op_tail.pop(0))
        for st in range(ROWS // P):
            op_job(st, NPAIR - 1, stop=True)
            ot = small.tile([P, D], f32, tag=f"ot{st % 2}", name=f"ot{st}")
            nc.vector.tensor_add(ot, op_tile(st), bob_sb)
            nc.sync.dma_start(out=out_d[st], in_=ot)

    nc.compile()
    return nc


def _get_program():
    if "p" not in _prog:
        _prog["p"] = _build_program()
    return _prog["p"]


def _stage_inputs(queries, keys, values, wq, bq, wk, bk, wv, bv, wo, bo):
    """Host staging: transpose activations to [D, S], per-core shards.
    bk is accepted and ignored (softmax-invariant)."""
    h = np.float16
    qT = queries.transpose(0, 2, 1).astype(h)
    kT = keys.transpose(0, 2, 1).astype(h)
    vT = values.transpose(0, 2, 1).astype(h)

    def wstage(w):
        # [H, D, 64] -> [P, NDC, D]: out[p, c, j] = W[c*128+p, j] (concat heads)
        wf = np.concatenate([w[i] for i in range(H)], axis=1)  # [D, D]
        return np.ascontiguousarray(wf.reshape(NDC, P, D).transpose(1, 0, 2)).astype(h)

    wq_m = wstage(wq)
    wk_m = wstage(wk)
    wv_m = wstage(wv)
    wo2_m = np.ascontiguousarray(wo.reshape(NPAIR, P, D)).astype(h)
    bq_m = np.ascontiguousarray(bq.reshape(NPAIR, P).T.astype(np.float32))
    bv1 = np.ascontiguousarray(bv.reshape(1, D).astype(np.float32))
    bo1 = np.ascontiguousarray(bo.reshape(1, D).astype(np.float32))

    kt_b = [
        np.ascontiguousarray(kT[b].reshape(NDC, P, S // 512, 512).transpose(2, 1, 0, 3))
        for b in range(B)
    ]
    vt_b = [
        np.ascontiguousarray(
            vT[b].reshape(NDC, P, NTT // 4, 4, 128).transpose(2, 1, 0, 3, 4)
        )
        for b in range(B)
    ]
    in_maps = []
    for c in range(NCORES):
        b, r = c // 4, c % 4
        qt_c = np.ascontiguousarray(
            qT[b][:, r * ROWS : (r + 1) * ROWS].reshape(NDC, P, ROWS).transpose(1, 0, 2)
        )
        in_maps.append(
            {
                "qt": qt_c, "kt": kt_b[b], "vt": vt_b[b],
                "wq": wq_m, "wk": wk_m, "wv": wv_m, "wo2": wo2_m,
                "bq": bq_m, "bv1": bv1, "bo1": bo1,
            }
        )
    return in_maps


def run(trace=False, **inputs):
    from concourse.bass_utils import run_bass_kernel_spmd

    nc = _get_program()
    in_maps = _stage_inputs(**inputs)
    res = run_bass_kernel_spmd(nc, in_maps, core_ids=list(range(NCORES)), trace=trace)
    out = np.empty((B, S, D), np.float32)
    for c in range(NCORES):
        b, r = c // 4, c % 4
        out[b, r * ROWS : (r + 1) * ROWS, :] = res.results[c]["out"].reshape(ROWS, D)
    return out, res


def kernel(**inputs):
    out, _ = run(trace=False, **inputs)
    return out
